# revision 31
# baseline (speedup 1.0000x reference)
"""MixLinear GEMM kernel for Trainium2 (8 NeuronCores, column-parallel).

Computes, for full inputs:
    inputs = x.reshape(-1, 4096)
    act_outliers = inputs[:, ind]
    inputs_z = inputs with ind-columns zeroed
    x_scale = clamp(rowmax(|inputs_z|)/127, 1e-8)
    q_x = round(inputs_z / x_scale)                  (|q_x| <= 127 by construction)
    y = (q_x @ q_weight.T) * x_scale * scale_col + act_outliers @ weight_cache.T + bias

Device-side formulation: the host pre-packs a combined bf16 weight
    Wc[k, o] = q_weight[o, k] * scale_col[o]          for k not in ind
    Wc[k, o] = sum_{j: ind[j]==k} weight_cache[o, j]  for k in ind
so that with q~[m, k] = round(x[m, k] / xs[m]) (UNMASKED - outlier columns
carry the rounded outlier activation, the same approximation the previous
baseline made) the output is simply
    y[m, o] = (sum_k q~[m, k] * Wc[k, o]) * xs[m] + bias[o].

q~ is produced in two ACT passes: x*recip + 1536 stored to fp16 (RNE
rounds to an exact integer: fp16 spacing is 1.0 in [1024, 2048)), then
-1536 stored to bf16 (integers up to +-256 are exact in bf16, and q~
stays well inside that). The GEMM then runs fully in bf16.

Sharding: Wc/bias are sharded along out_features across the 8 cores
(column parallel); x and the ind-mask are replicated. Each core produces
its (512, 1376) output shard; the host concatenates.

Schedule: emission is software-pipelined (phase1(r+1) before phase2(r))
with q double-buffered across reps, so quantization of rep r+1 overlaps
the GEMM of rep r. Engine queues: SP carries x loads + q transposes
(loads are emitted one m-tile ahead, always before the transposes, so a
load never queues behind one); ACT does the quantize passes + y stores;
DVE does masking/absmax/output scaling; Pool (gpsimd) streams the
weights; PE does the matmuls with all 8 PSUM banks in flight.
"""

import sys

import numpy as np

sys.path.insert(0, "/opt/trn_rl_repo")

import concourse.bass as bass  # noqa: E402
import concourse.mybir as mybir  # noqa: E402
import concourse.tile as tile  # noqa: E402
from concourse import bacc  # noqa: E402

N_CORES = 8
M = 512  # 8*64 rows
K = 4096  # in_features
OUT = 11008  # out_features
OSH = OUT // N_CORES  # 1376 per-core shard
FP = 256  # outlier columns
KT = K // 128  # 32 k-tiles
MT = M // 128  # 4 m-tiles
MAGIC = 1536.0  # fp16 spacing is 1.0 in [1024, 2048): forces round-to-int
OC = 459  # padded o-chunk width (fits one PSUM bank: 459*4B <= 2KB)
OCS = [459, 459, 458]  # actual chunk widths (sum = OSH)
OFF = [0, 459, 918]
NCH = 3  # chunks
XH = 2048  # x streamed in half-rows
XQ = 1024  # absmax computed in chunks of this width

f32 = mybir.dt.float32
f16 = mybir.dt.float16
bf16 = mybir.dt.bfloat16
Alu = mybir.AluOpType
Act = mybir.ActivationFunctionType


def build_program(nrep=1, debug_dump=False):
    """Build the kernel program. nrep>1 emits the whole body nrep times
    (same inputs, same outputs) — used only to measure steady-state HW time
    as (t(nrep) - t(1)) / (nrep - 1)."""
    nc = bacc.Bacc(
        "TRN2", target_bir_lowering=False, debug=False, num_devices=N_CORES
    )

    x_d = nc.dram_tensor("x_in", [M, K], f16, kind="ExternalInput").ap()
    # host-packed combined weight: [chunk, partition(k%128), kk, o-in-chunk]
    w_d = nc.dram_tensor("w_in", [NCH, 128, KT * OC], bf16, kind="ExternalInput").ap()
    mask_d = nc.dram_tensor("mask_in", [1, K], f16, kind="ExternalInput").ap()
    bias_d = nc.dram_tensor("bias_in", [1, OSH], bf16, kind="ExternalInput").ap()
    y_d = nc.dram_tensor("y_out", [M, OSH], f32, kind="ExternalOutput").ap()
    dbg = {}
    if debug_dump:
        for nm, shape, dt in [
            ("dbg_scales", [128, 4 * MT], f32),
            ("dbg_q0", [128, KT * 128], bf16),
            ("dbg_wt0", [128, KT * OC], bf16),
        ]:
            dbg[nm] = nc.dram_tensor(nm, shape, dt, kind="ExternalOutput").ap()

    with tile.TileContext(nc) as tc:
        with (
            tc.tile_pool(name="persist", bufs=1) as persist,
            tc.tile_pool(name="xpool", bufs=4) as xpool,
            tc.tile_pool(name="xzpool", bufs=2) as xzpool,
            tc.tile_pool(name="qnpool", bufs=2) as qnpool,
            tc.tile_pool(name="qbpool", bufs=2) as qbpool,
            tc.tile_pool(name="wtpool", bufs=3) as wtpool,
            tc.tile_pool(name="ypool", bufs=3) as ypool,
            tc.tile_pool(name="psmain", bufs=8, space="PSUM") as psmain,
        ):
            # ---------- persistent tiles ----------
            # q^T (k-part, kk, m): one tile per m-tile, double-buffered
            # across reps so rep r+1's quantization overlaps rep r's GEMM.
            q_sets = []
            for par in range(2):
                qset = []
                for mt in range(MT):
                    q_t = persist.tile(
                        [128, KT, 128],
                        bf16,
                        tag=f"qT{par}_{mt}",
                        name=f"qT{par}_{mt}",
                    )
                    qset.append(q_t)
                q_sets.append(qset)
            mask_bc = persist.tile([128, K], f16)  # ind-mask broadcast
            bias_bc = persist.tile([128, OSH], bf16)  # bias broadcast
            am_parts = persist.tile([128, MT * (K // XQ)], f32)
            am_all = persist.tile([128, MT], f32)
            xs_all = persist.tile([128, 4 * MT], f32)  # rep%4-indexed
            recip_all = persist.tile([128, 4 * MT], f32)

            # ---------- setup ----------
            nc.gpsimd.dma_start(
                out=mask_bc,
                in_=bass.AP(mask_d.tensor, mask_d.offset, [[0, 128], [1, K]]),
            )
            nc.gpsimd.dma_start(
                out=bias_bc,
                in_=bass.AP(bias_d.tensor, bias_d.offset, [[0, 128], [1, OSH]]),
            )

            def phase1(rep):
                par = rep % 2
                pq = rep % 4
                q_tiles = q_sets[par]
                nhalf = K // XH  # 2
                nq = XH // XQ  # 2
                def load_x(mt):
                    # x loads issue from SP, but are emitted one m-tile
                    # ahead — after the previous m-tile's activations (the
                    # last readers of the buffer being recycled) and BEFORE
                    # its transposes, so a load never queues behind a
                    # transpose (which would serialize phase 1 mt-by-mt).
                    ms = slice(mt * 128, (mt + 1) * 128)
                    tiles = []
                    for h in range(nhalf):
                        x_h = xpool.tile(
                            [128, XH], f16, tag="x", name=f"x_{rep}_{mt}_{h}"
                        )
                        nc.sync.dma_start(
                            out=x_h, in_=x_d[ms, h * XH : (h + 1) * XH]
                        )
                        tiles.append(x_h)
                    return tiles

                x_cur = load_x(0)
                for mt in range(MT):
                    ms = slice(mt * 128, (mt + 1) * 128)
                    x_hs = []
                    for h in range(nhalf):
                        x_h = x_cur[h]
                        x_hs.append(x_h)
                        for q in range(nq):
                            xz = xzpool.tile(
                                [128, XQ], f16, tag="xz", name=f"xz_{rep}_{mt}_{h}_{q}"
                            )
                            nc.vector.tensor_tensor(
                                out=xz,
                                in0=x_h[:, q * XQ : (q + 1) * XQ],
                                in1=mask_bc[
                                    :, (h * nq + q) * XQ : (h * nq + q + 1) * XQ
                                ],
                                op=Alu.mult,
                            )
                            pcol = mt * (K // XQ) + h * nq + q
                            nc.vector.tensor_reduce(
                                out=am_parts[:, pcol : pcol + 1],
                                in_=xz,
                                axis=mybir.AxisListType.X,
                                op=Alu.max,
                                apply_absolute_value=True,
                            )
                    nc.vector.tensor_reduce(
                        out=am_all[:, mt : mt + 1],
                        in_=am_parts[:, mt * (K // XQ) : (mt + 1) * (K // XQ)],
                        axis=mybir.AxisListType.X,
                        op=Alu.max,
                        apply_absolute_value=False,
                    )
                    pc = pq * MT + mt
                    # xs = max(absmax/127, 1e-8); recip = 1/xs
                    nc.vector.tensor_scalar(
                        xs_all[:, pc : pc + 1],
                        am_all[:, mt : mt + 1],
                        1.0 / 127.0,
                        1e-8,
                        Alu.mult,
                        Alu.max,
                    )
                    nc.vector.reciprocal(
                        out=recip_all[:, pc : pc + 1], in_=xs_all[:, pc : pc + 1]
                    )
                    q_t = q_tiles[mt]
                    qbs = []
                    for h in range(nhalf):
                        # pass 1: x*recip + 1536 -> fp16 write rounds to
                        # int (RNE); pass 2: subtract the magic, store the
                        # exact small ints as bf16 for the GEMM
                        qn = qnpool.tile(
                            [128, XH], f16, tag="qn", name=f"qn_{rep}_{mt}_{h}"
                        )
                        nc.scalar.activation(
                            out=qn,
                            in_=x_hs[h],
                            func=Act.Copy,
                            bias=MAGIC,
                            scale=recip_all[:, pc : pc + 1],
                        )
                        qb = qbpool.tile(
                            [128, XH], bf16, tag="qb", name=f"qb_{rep}_{mt}_{h}"
                        )
                        # second pass on DVE (16-bit, 2x rate) so the ACT
                        # queue only carries one activation per half
                        nc.vector.tensor_scalar(
                            qb, qn, -MAGIC, None, Alu.add
                        )
                        qbs.append(qb)
                    if mt + 1 < MT:
                        x_cur = load_x(mt + 1)
                    for h in range(nhalf):
                        # transpose into q_t[:, k-half, :].
                        # NOTE: dma transpose must be issued from the SP
                        # sequencer — ACT-issued xbar transposes corrupt
                        # data on HW.
                        nc.sync.dma_start(
                            out=q_t[:, h * (XH // 128) : (h + 1) * (XH // 128), :],
                            in_=qbs[h],
                            transpose=True,
                        )

            def load_w(rep, c):
                wt = wtpool.tile(
                    [128, KT, OC], bf16, tag="wt", name=f"wt_{rep}_{c}"
                )
                nc.gpsimd.dma_start(out=wt, in_=w_d[c])
                return wt

            wt_next = {}

            def phase2(rep, prefetch_next):
                par = rep % 2
                pq = rep % 4
                q_tiles = q_sets[par]
                nonlocal wt_next
                wt_cur = wt_next if wt_next else {c: load_w(rep, c) for c in range(NCH)}
                wt_next = {}
                for c in range(NCH):
                    wt = wt_cur[c]
                    o0 = OFF[c]
                    cw = OCS[c]
                    for mt in range(MT):
                        ms = slice(mt * 128, (mt + 1) * 128)
                        pc = pq * MT + mt
                        ps = psmain.tile(
                            [128, OC], f32, tag="ps", name=f"ps_{rep}_{c}_{mt}"
                        )
                        for kk in range(KT):
                            nc.tensor.matmul(
                                ps,
                                lhsT=q_tiles[mt][:, kk, :],
                                rhs=wt[:, kk, :],
                                start=(kk == 0),
                                stop=(kk == KT - 1),
                            )
                        ysb = ypool.tile(
                            [128, OC], f32, tag="ysb", name=f"ysb_{rep}_{c}_{mt}"
                        )
                        # y = ps * xs + bias
                        nc.vector.scalar_tensor_tensor(
                            out=ysb[:, :cw],
                            in0=ps[:, :cw],
                            scalar=xs_all[:, pc : pc + 1],
                            in1=bias_bc[:, o0 : o0 + cw],
                            op0=Alu.mult,
                            op1=Alu.add,
                        )
                        nc.scalar.dma_start(
                            out=y_d[ms, o0 : o0 + cw], in_=ysb[:, :cw]
                        )
                    # prefetch next rep's chunk-c weights now that this
                    # rep's reads of the same wt buffer are emitted
                    if prefetch_next:
                        wt_next[c] = load_w(rep + 1, c)
                if debug_dump and rep == 0:
                    nc.sync.dma_start(out=dbg["dbg_scales"], in_=xs_all)
                    nc.sync.dma_start(out=dbg["dbg_q0"], in_=q_tiles[0][:, :, :])
                    nc.sync.dma_start(out=dbg["dbg_wt0"], in_=wt_cur[0][:, :, :])

            # software-pipelined emission: phase1(r+1) before phase2(r)
            phase1(0)
            for rep in range(1, nrep):
                phase1(rep)
                phase2(rep - 1, prefetch_next=(rep < nrep))
            phase2(nrep - 1, prefetch_next=False)

    nc.compile()
    return nc


_NC_CACHE = None


def get_program():
    global _NC_CACHE
    if _NC_CACHE is None:
        _NC_CACHE = build_program()
    return _NC_CACHE


def make_in_maps(x, q_weight, scale_col, weight_cache, ind, bias):
    x2 = np.ascontiguousarray(np.asarray(x, dtype=np.float32).reshape(M, K).astype(np.float16))
    q_weight = np.asarray(q_weight, dtype=np.int32)
    scale_col = np.asarray(scale_col, dtype=np.float32).reshape(OUT)
    weight_cache = np.asarray(weight_cache, dtype=np.float32)
    ind_np = np.asarray(ind, dtype=np.int32).reshape(FP)
    bias_np = np.asarray(bias, dtype=np.float32).reshape(OUT)

    import ml_dtypes

    mask = np.ones(K, dtype=np.float32)
    mask[ind_np] = 0.0
    mask_bf = mask.astype(np.float16).reshape(1, K)

    # combined weight: WcT[k, o] = q_weight[o, k]*scale_col[o] off-outlier,
    # scatter-add of weight_cache on outlier rows (duplicates in ind add,
    # matching x[:, ind] gather + separate GEMM in the reference)
    wf = q_weight.astype(np.float32) * scale_col.reshape(OUT, 1)  # [OUT, K]
    wcT = np.ascontiguousarray(wf.T)  # [K, OUT]
    cr = np.zeros((K, OUT), dtype=np.float32)
    np.add.at(cr, ind_np, weight_cache.T.astype(np.float32))
    outlier_rows = np.zeros(K, dtype=bool)
    outlier_rows[ind_np] = True
    wcT[outlier_rows] = cr[outlier_rows]
    wc16 = wcT.astype(ml_dtypes.bfloat16)  # [K, OUT]

    in_maps = []
    for c in range(N_CORES):
        sl = slice(c * OSH, (c + 1) * OSH)
        shard = wc16[:, sl]  # [K, OSH]
        # pack: [chunk, partition(k%128), kk, o-in-chunk], zero-padded to OC
        wpack = np.zeros((NCH, 128, KT, OC), dtype=wc16.dtype)
        r = shard.reshape(KT, 128, OSH)
        for c in range(NCH):
            wpack[c, :, :, : OCS[c]] = r[:, :, OFF[c] : OFF[c] + OCS[c]].transpose(
                1, 0, 2
            )
        wpack = np.ascontiguousarray(wpack).reshape(NCH, 128, KT * OC)
        in_maps.append(
            {
                "x_in": x2,
                "w_in": wpack,
                "mask_in": mask_bf,
                "bias_in": np.ascontiguousarray(
                    bias_np[sl].astype(ml_dtypes.bfloat16).reshape(1, OSH)
                ),
            }
        )
    return in_maps


def kernel(x, q_weight, scale_col, weight_cache, ind, bias):
    from concourse.bass_utils import run_bass_kernel_spmd

    nc = get_program()
    in_maps = make_in_maps(x, q_weight, scale_col, weight_cache, ind, bias)
    res = run_bass_kernel_spmd(nc, in_maps, core_ids=list(range(N_CORES)))
    shards = [res.results[c]["y_out"] for c in range(N_CORES)]
    y = np.concatenate(shards, axis=1)
    return y.reshape(8, 64, OUT).astype(np.float32)


# revision 33
# speedup vs baseline: 1.0161x; 1.0161x over previous
"""MixLinear GEMM kernel for Trainium2 (8 NeuronCores, column-parallel).

Computes, for full inputs:
    inputs = x.reshape(-1, 4096)
    act_outliers = inputs[:, ind]
    inputs_z = inputs with ind-columns zeroed
    x_scale = clamp(rowmax(|inputs_z|)/127, 1e-8)
    q_x = round(inputs_z / x_scale)                  (|q_x| <= 127 by construction)
    y = (q_x @ q_weight.T) * x_scale * scale_col + act_outliers @ weight_cache.T + bias

Device-side formulation: the host pre-packs a combined bf16 weight
    Wc[k, o] = q_weight[o, k] * scale_col[o]          for k not in ind
    Wc[k, o] = sum_{j: ind[j]==k} weight_cache[o, j]  for k in ind
so that with q~[m, k] = round(x[m, k] / xs[m]) (UNMASKED - outlier columns
carry the rounded outlier activation, the same approximation the previous
baseline made) the output is simply
    y[m, o] = (sum_k q~[m, k] * Wc[k, o]) * xs[m] + bias[o].

q~ is produced in two ACT passes: x*recip + 1536 stored to fp16 (RNE
rounds to an exact integer: fp16 spacing is 1.0 in [1024, 2048)), then
-1536 stored to bf16 (integers up to +-256 are exact in bf16, and q~
stays well inside that). The GEMM then runs fully in bf16.

Sharding: Wc/bias are sharded along out_features across the 8 cores
(column parallel); x and the ind-mask are replicated. Each core produces
its (512, 1376) output shard; the host concatenates.

Schedule: emission is software-pipelined (phase1(r+1) before phase2(r))
with q double-buffered across reps, so quantization of rep r+1 overlaps
the GEMM of rep r. Engine queues: SP carries x loads + q transposes
(loads are emitted one m-tile ahead, always before the transposes, so a
load never queues behind one); ACT does the quantize passes + y stores;
DVE does masking/absmax/output scaling; Pool (gpsimd) streams the
weights; PE does the matmuls with all 8 PSUM banks in flight.
"""

import sys

import numpy as np

sys.path.insert(0, "/opt/trn_rl_repo")

import concourse.bass as bass  # noqa: E402
import concourse.mybir as mybir  # noqa: E402
import concourse.tile as tile  # noqa: E402
from concourse import bacc  # noqa: E402

N_CORES = 8
M = 512  # 8*64 rows
K = 4096  # in_features
OUT = 11008  # out_features
OSH = OUT // N_CORES  # 1376 per-core shard
FP = 256  # outlier columns
KT = K // 128  # 32 k-tiles
MT = M // 128  # 4 m-tiles
MAGIC = 1536.0  # fp16 spacing is 1.0 in [1024, 2048): forces round-to-int
OC = 459  # padded o-chunk width (fits one PSUM bank: 459*4B <= 2KB)
OCS = [459, 459, 458]  # actual chunk widths (sum = OSH)
OFF = [0, 459, 918]
NCH = 3  # chunks
XH = 2048  # x streamed in half-rows
XQ = 1024  # absmax computed in chunks of this width

f32 = mybir.dt.float32
f16 = mybir.dt.float16
bf16 = mybir.dt.bfloat16
Alu = mybir.AluOpType
Act = mybir.ActivationFunctionType


def build_program(nrep=1, debug_dump=False):
    """Build the kernel program. nrep>1 emits the whole body nrep times
    (same inputs, same outputs) — used only to measure steady-state HW time
    as (t(nrep) - t(1)) / (nrep - 1)."""
    nc = bacc.Bacc(
        "TRN2", target_bir_lowering=False, debug=False, num_devices=N_CORES
    )

    x_d = nc.dram_tensor("x_in", [M, K], f16, kind="ExternalInput").ap()
    # host-packed combined weight: [chunk, partition(k%128), kk, o-in-chunk]
    w_d = nc.dram_tensor("w_in", [NCH, 128, KT * OC], bf16, kind="ExternalInput").ap()
    mask_d = nc.dram_tensor("mask_in", [1, K], f16, kind="ExternalInput").ap()
    bias_d = nc.dram_tensor("bias_in", [1, OSH], bf16, kind="ExternalInput").ap()
    y_d = nc.dram_tensor("y_out", [M, OSH], f32, kind="ExternalOutput").ap()
    dbg = {}
    if debug_dump:
        for nm, shape, dt in [
            ("dbg_scales", [128, 4 * MT], f32),
            ("dbg_q0", [128, KT * 128], bf16),
            ("dbg_wt0", [128, KT * OC], bf16),
        ]:
            dbg[nm] = nc.dram_tensor(nm, shape, dt, kind="ExternalOutput").ap()

    with tile.TileContext(nc) as tc:
        with (
            tc.tile_pool(name="persist", bufs=1) as persist,
            tc.tile_pool(name="xpool", bufs=4) as xpool,
            tc.tile_pool(name="xzpool", bufs=2) as xzpool,
            tc.tile_pool(name="qnpool", bufs=2) as qnpool,
            tc.tile_pool(name="qbpool", bufs=3) as qbpool,
            tc.tile_pool(name="wtpool", bufs=3) as wtpool,
            tc.tile_pool(name="ypool", bufs=3) as ypool,
            tc.tile_pool(name="psmain", bufs=8, space="PSUM") as psmain,
        ):
            # ---------- persistent tiles ----------
            # q^T (k-part, kk, m): one tile per m-tile, double-buffered
            # across reps so rep r+1's quantization overlaps rep r's GEMM.
            q_sets = []
            for par in range(2):
                qset = []
                for mt in range(MT):
                    q_t = persist.tile(
                        [128, KT, 128],
                        bf16,
                        tag=f"qT{par}_{mt}",
                        name=f"qT{par}_{mt}",
                    )
                    qset.append(q_t)
                q_sets.append(qset)
            mask_bc = persist.tile([128, K], f16)  # ind-mask broadcast
            bias_bc = persist.tile([128, OSH], bf16)  # bias broadcast
            am_parts = persist.tile([128, MT * (K // XQ)], f32)
            am_all = persist.tile([128, MT], f32)
            xs_all = persist.tile([128, 4 * MT], f32)  # rep%4-indexed
            recip_all = persist.tile([128, 4 * MT], f32)

            # ---------- setup ----------
            nc.gpsimd.dma_start(
                out=mask_bc,
                in_=bass.AP(mask_d.tensor, mask_d.offset, [[0, 128], [1, K]]),
            )
            nc.gpsimd.dma_start(
                out=bias_bc,
                in_=bass.AP(bias_d.tensor, bias_d.offset, [[0, 128], [1, OSH]]),
            )

            def phase1(rep):
                par = rep % 2
                pq = rep % 4
                q_tiles = q_sets[par]
                nhalf = K // XH  # 2
                nq = XH // XQ  # 2
                def load_x(mt):
                    # x loads issue from SP, but are emitted one m-tile
                    # ahead — after the previous m-tile's activations (the
                    # last readers of the buffer being recycled) and BEFORE
                    # its transposes, so a load never queues behind a
                    # transpose (which would serialize phase 1 mt-by-mt).
                    ms = slice(mt * 128, (mt + 1) * 128)
                    tiles = []
                    for h in range(nhalf):
                        x_h = xpool.tile(
                            [128, XH], f16, tag="x", name=f"x_{rep}_{mt}_{h}"
                        )
                        nc.sync.dma_start(
                            out=x_h, in_=x_d[ms, h * XH : (h + 1) * XH]
                        )
                        tiles.append(x_h)
                    return tiles

                x_cur = load_x(0)
                for mt in range(MT):
                    ms = slice(mt * 128, (mt + 1) * 128)
                    x_hs = []
                    for h in range(nhalf):
                        x_h = x_cur[h]
                        x_hs.append(x_h)
                        for q in range(nq):
                            xz = xzpool.tile(
                                [128, XQ], f16, tag="xz", name=f"xz_{rep}_{mt}_{h}_{q}"
                            )
                            nc.vector.tensor_tensor(
                                out=xz,
                                in0=x_h[:, q * XQ : (q + 1) * XQ],
                                in1=mask_bc[
                                    :, (h * nq + q) * XQ : (h * nq + q + 1) * XQ
                                ],
                                op=Alu.mult,
                            )
                            pcol = mt * (K // XQ) + h * nq + q
                            nc.vector.tensor_reduce(
                                out=am_parts[:, pcol : pcol + 1],
                                in_=xz,
                                axis=mybir.AxisListType.X,
                                op=Alu.max,
                                apply_absolute_value=True,
                            )
                    nc.vector.tensor_reduce(
                        out=am_all[:, mt : mt + 1],
                        in_=am_parts[:, mt * (K // XQ) : (mt + 1) * (K // XQ)],
                        axis=mybir.AxisListType.X,
                        op=Alu.max,
                        apply_absolute_value=False,
                    )
                    pc = pq * MT + mt
                    # xs = max(absmax/127, 1e-8); recip = 1/xs
                    nc.vector.tensor_scalar(
                        xs_all[:, pc : pc + 1],
                        am_all[:, mt : mt + 1],
                        1.0 / 127.0,
                        1e-8,
                        Alu.mult,
                        Alu.max,
                    )
                    nc.vector.reciprocal(
                        out=recip_all[:, pc : pc + 1], in_=xs_all[:, pc : pc + 1]
                    )
                    q_t = q_tiles[mt]
                    qbs = []
                    for h in range(nhalf):
                        # pass 1: x*recip + 1536 -> fp16 write rounds to
                        # int (RNE); pass 2: subtract the magic, store the
                        # exact small ints as bf16 for the GEMM
                        qn = qnpool.tile(
                            [128, XH], f16, tag="qn", name=f"qn_{rep}_{mt}_{h}"
                        )
                        nc.scalar.activation(
                            out=qn,
                            in_=x_hs[h],
                            func=Act.Copy,
                            bias=MAGIC,
                            scale=recip_all[:, pc : pc + 1],
                        )
                        qb = qbpool.tile(
                            [128, XH], bf16, tag="qb", name=f"qb_{rep}_{mt}_{h}"
                        )
                        nc.scalar.activation(
                            out=qb,
                            in_=qn,
                            func=Act.Copy,
                            bias=-MAGIC,
                            scale=1.0,
                        )
                        qbs.append(qb)
                    if mt + 1 < MT:
                        x_cur = load_x(mt + 1)
                    for h in range(nhalf):
                        # transpose into q_t[:, k-half, :].
                        # NOTE: dma transpose must be issued from the SP
                        # sequencer — ACT-issued xbar transposes corrupt
                        # data on HW.
                        nc.sync.dma_start(
                            out=q_t[:, h * (XH // 128) : (h + 1) * (XH // 128), :],
                            in_=qbs[h],
                            transpose=True,
                        )

            def load_w(rep, c):
                wt = wtpool.tile(
                    [128, KT, OC], bf16, tag="wt", name=f"wt_{rep}_{c}"
                )
                nc.gpsimd.dma_start(out=wt, in_=w_d[c])
                return wt

            wt_next = {}

            def phase2(rep, prefetch_next):
                par = rep % 2
                pq = rep % 4
                q_tiles = q_sets[par]
                nonlocal wt_next
                wt_cur = wt_next if wt_next else {c: load_w(rep, c) for c in range(NCH)}
                wt_next = {}
                for c in range(NCH):
                    wt = wt_cur[c]
                    o0 = OFF[c]
                    cw = OCS[c]
                    for mt in range(MT):
                        ms = slice(mt * 128, (mt + 1) * 128)
                        pc = pq * MT + mt
                        ps = psmain.tile(
                            [128, OC], f32, tag="ps", name=f"ps_{rep}_{c}_{mt}"
                        )
                        for kk in range(KT):
                            nc.tensor.matmul(
                                ps,
                                lhsT=q_tiles[mt][:, kk, :],
                                rhs=wt[:, kk, :],
                                start=(kk == 0),
                                stop=(kk == KT - 1),
                            )
                        ysb = ypool.tile(
                            [128, OC], f32, tag="ysb", name=f"ysb_{rep}_{c}_{mt}"
                        )
                        # y = ps * xs + bias
                        nc.vector.scalar_tensor_tensor(
                            out=ysb[:, :cw],
                            in0=ps[:, :cw],
                            scalar=xs_all[:, pc : pc + 1],
                            in1=bias_bc[:, o0 : o0 + cw],
                            op0=Alu.mult,
                            op1=Alu.add,
                        )
                        nc.scalar.dma_start(
                            out=y_d[ms, o0 : o0 + cw], in_=ysb[:, :cw]
                        )
                    # prefetch next rep's chunk-c weights now that this
                    # rep's reads of the same wt buffer are emitted
                    if prefetch_next:
                        wt_next[c] = load_w(rep + 1, c)
                if debug_dump and rep == 0:
                    nc.sync.dma_start(out=dbg["dbg_scales"], in_=xs_all)
                    nc.sync.dma_start(out=dbg["dbg_q0"], in_=q_tiles[0][:, :, :])
                    nc.sync.dma_start(out=dbg["dbg_wt0"], in_=wt_cur[0][:, :, :])

            # software-pipelined emission: phase1(r+1) before phase2(r)
            phase1(0)
            for rep in range(1, nrep):
                phase1(rep)
                phase2(rep - 1, prefetch_next=(rep < nrep))
            phase2(nrep - 1, prefetch_next=False)

    nc.compile()
    return nc


_NC_CACHE = None


def get_program():
    global _NC_CACHE
    if _NC_CACHE is None:
        _NC_CACHE = build_program()
    return _NC_CACHE


def make_in_maps(x, q_weight, scale_col, weight_cache, ind, bias):
    x2 = np.ascontiguousarray(np.asarray(x, dtype=np.float32).reshape(M, K).astype(np.float16))
    q_weight = np.asarray(q_weight, dtype=np.int32)
    scale_col = np.asarray(scale_col, dtype=np.float32).reshape(OUT)
    weight_cache = np.asarray(weight_cache, dtype=np.float32)
    ind_np = np.asarray(ind, dtype=np.int32).reshape(FP)
    bias_np = np.asarray(bias, dtype=np.float32).reshape(OUT)

    import ml_dtypes

    mask = np.ones(K, dtype=np.float32)
    mask[ind_np] = 0.0
    mask_bf = mask.astype(np.float16).reshape(1, K)

    # combined weight: WcT[k, o] = q_weight[o, k]*scale_col[o] off-outlier,
    # scatter-add of weight_cache on outlier rows (duplicates in ind add,
    # matching x[:, ind] gather + separate GEMM in the reference)
    wf = q_weight.astype(np.float32) * scale_col.reshape(OUT, 1)  # [OUT, K]
    wcT = np.ascontiguousarray(wf.T)  # [K, OUT]
    cr = np.zeros((K, OUT), dtype=np.float32)
    np.add.at(cr, ind_np, weight_cache.T.astype(np.float32))
    outlier_rows = np.zeros(K, dtype=bool)
    outlier_rows[ind_np] = True
    wcT[outlier_rows] = cr[outlier_rows]
    wc16 = wcT.astype(ml_dtypes.bfloat16)  # [K, OUT]

    in_maps = []
    for c in range(N_CORES):
        sl = slice(c * OSH, (c + 1) * OSH)
        shard = wc16[:, sl]  # [K, OSH]
        # pack: [chunk, partition(k%128), kk, o-in-chunk], zero-padded to OC
        wpack = np.zeros((NCH, 128, KT, OC), dtype=wc16.dtype)
        r = shard.reshape(KT, 128, OSH)
        for c in range(NCH):
            wpack[c, :, :, : OCS[c]] = r[:, :, OFF[c] : OFF[c] + OCS[c]].transpose(
                1, 0, 2
            )
        wpack = np.ascontiguousarray(wpack).reshape(NCH, 128, KT * OC)
        in_maps.append(
            {
                "x_in": x2,
                "w_in": wpack,
                "mask_in": mask_bf,
                "bias_in": np.ascontiguousarray(
                    bias_np[sl].astype(ml_dtypes.bfloat16).reshape(1, OSH)
                ),
            }
        )
    return in_maps


def kernel(x, q_weight, scale_col, weight_cache, ind, bias):
    from concourse.bass_utils import run_bass_kernel_spmd

    nc = get_program()
    in_maps = make_in_maps(x, q_weight, scale_col, weight_cache, ind, bias)
    res = run_bass_kernel_spmd(nc, in_maps, core_ids=list(range(N_CORES)))
    shards = [res.results[c]["y_out"] for c in range(N_CORES)]
    y = np.concatenate(shards, axis=1)
    return y.reshape(8, 64, OUT).astype(np.float32)
